# revision 2
# baseline (speedup 1.0000x reference)
"""Trainium2 Bass kernel for nn_DecisionMaker (retrieval_knn), v3.

v3 = v2 + direction-split GRU across core pairs.

The GRU's whh matmuls are weight-ingestion bound on the PE (the full
512x1536 whh must stream through the array every step regardless of
batch), so data-parallel batch sharding gains nothing there. v3 instead
splits by DIRECTION: cores 0-3 run the forward GRU, cores 4-7 the
backward GRU, each over the 32 batch rows of its core pair (c, c+4).
That halves whh (102->51us) and selector (27->13.4us) PE time per core.
After the scan, a pairwise AllGather ([ [0,4],[1,5],[2,6],[3,7] ])
exchanges the 32x512 final hidden states; each core then picks its own
16 rows of h_f and h_b via an indirect row gather (SPMD-safe: the row
indices are per-core input data) and PE-transposes them for the ctx
matmuls. The exchange is issued right after the GRU and hidden behind
the uf/CNN PE work, which does not depend on h.

Everything else stays per-core data-parallel over the core's own 16
batch rows, as in v2: feature-on-partition layouts, tb = t*16+b
(t-major, TB=336); the GRU uses tb2 = t*32+b2 (TB2=672, 6 chunks).
"""

import functools
from collections import deque

import numpy as np
import ml_dtypes

import concourse.bass as bass
import concourse.tile as tile
from concourse import bacc, mybir
from concourse.bass_utils import run_bass_kernel_spmd

F32 = mybir.dt.float32
BF16 = mybir.dt.bfloat16
AF = mybir.ActivationFunctionType
OP = mybir.AluOpType
BF = ml_dtypes.bfloat16

NCORE = 8
B, T, K, E, H, IC = 128, 21, 6, 512, 512, 2048
CV = 10001
BPC = B // NCORE            # 16 batch rows per core (data-parallel parts)
PB = 2 * BPC                # 32 batch rows per GRU pair
TB = T * BPC                # 336 (t-major: tb = t*16 + b)
TBP = 384                   # padded to 3*128 for gathers
NCH = 3                     # tb chunks of 128
TB2 = T * PB                # 672 (GRU: tb2 = t*32 + b2)
NCH2 = 6                    # tb2 chunks of 128 (672 -> 768 padded)
TBP2 = NCH2 * 128
G3 = 3 * H                  # 1536 gate width
H2 = 2 * H
PAIRS = [(i, j) for i in range(K) for j in range(i + 1, K)]  # 15, (0,k) first
# gate column slices in the PyTorch order r, z, n
SL_R = slice(0, H)
SL_Z = slice(H, H2)
SL_N = slice(H2, G3)


class Filler:
    """Credit-based queue of (cost_ns, emit_fn) items drained into gaps."""

    def __init__(self):
        self.q = deque()
        self.credit = 0.0

    def add(self, cost, fn):
        self.q.append((cost, fn))

    def step(self, budget):
        self.credit += budget
        while self.q and self.q[0][0] <= self.credit:
            cost, fn = self.q.popleft()
            fn()
            self.credit -= cost

    def flush(self):
        while self.q:
            self.q.popleft()[1]()
        self.credit = 0.0


def _emit(nc, tc, io, stage=99):
    from contextlib import ExitStack
    ctx = ExitStack()
    wp = ctx.enter_context(tc.tile_pool(name="weights", bufs=1))
    ap_ = ctx.enter_context(tc.tile_pool(name="acts", bufs=1))
    gp = ctx.enter_context(tc.tile_pool(name="gather", bufs=1))
    sp = ctx.enter_context(tc.tile_pool(name="scratch", bufs=2))
    fp = ctx.enter_context(tc.tile_pool(name="feats", bufs=1))
    ps_ = ctx.enter_context(tc.tile_pool(name="psum", bufs=1, space="PSUM"))
    dr = ctx.enter_context(tc.tile_pool(name="dram", bufs=1, space="DRAM"))
    # stage 98 = 8x full body; 97 = 16x; 96 = 24x; 900+S = 8x stage-S body
    nrep, body = {98: (8, 99), 97: (16, 99), 96: (24, 99)}.get(
        stage, (8, stage - 900) if stage >= 900 else (1, stage))
    for _rep in range(nrep):
        _emit_body(nc, tc, io, body, ctx, wp, ap_, gp, sp, fp, ps_, dr)
    ctx.close()


def _emit_body(nc, tc, io, stage, ctx, wp, ap_, gp, sp, fp, ps_, dr):

    # ---------------- resident weights (bf16, host pre-arranged so every
    # multi-chunk tensor loads with ONE contiguous partition-major DMA)
    def load_one(name, shape, dt=BF16, pool=None):
        t = (pool or wp).tile(list(shape), dt, tag=name, name=name)
        nc.sync.dma_start(t[:], io[name][:].rearrange(
            "p (k n) -> p k n", k=shape[1]) if len(shape) == 3 else io[name][:])
        return t

    def chunks(t, n):
        return [t[:, k, :] for k in range(n)]

    # gathers first: they feed the critical gi -> GRU path
    # (one SWDGE call per gidx column -- multi-column offset APs corrupt)
    gidx = gp.tile([128, 25], mybir.dt.int32, tag="gidx")
    nc.sync.dma_start(gidx[:], io["gidx"][:])

    def gather_rows(dst, table, col, nrow=128):
        nc.gpsimd.indirect_dma_start(
            out=dst, out_offset=None, in_=table,
            in_offset=bass.IndirectOffsetOnAxis(ap=gidx[0:nrow, col:col + 1],
                                                axis=0))

    # capeall: this core's GRU direction, 32 pair-batch rows, 6 chunks
    capeall = gp.tile([128, NCH2, E], BF16, tag="capeall", name="capeall")
    for j in range(NCH2):
        gather_rows(capeall[:, j, :], io["cap_emb_w_bf"][:], 18 + j)
    emball = gp.tile([128, K * NCH, E], BF16, tag="emball", name="emball")
    for c in range(NCH):          # c-major: the inline ttr tranche reads c=0
        for k in range(K):
            j = k * NCH + c
            gather_rows(emball[:, j, :], io["cap_emb_bf"][:], j)
    emb = [emball[:, k * NCH:(k + 1) * NCH, :] for k in range(K)]

    # weights in critical-path order: gi/GRU first, post-GRU (cWT) last
    ident = load_one("ident", [128, 128])          # bf16 identity
    smallp = load_one("smallp", [1, G3 + 128])     # ones row + gru bias (own dir)
    ones1 = smallp[0:1, 0:128]
    gbrow = smallp[0:1, 128:128 + G3]
    wih = chunks(load_one("wihT", [128, 4, G3]), 4)
    whh = chunks(load_one("whhT", [128, 4, G3]), 4)
    posW = load_one("posW", [51, 128])             # pos_emb_w, natural lhsT
    w1T = load_one("w1T", [12, 128])
    pWT = load_one("pWT", [128, 256])
    hWT = chunks(load_one("hWT", [128, 4, 256]), 4)
    w2T = chunks(load_one("w2T", [128, 3, 256]), 3)
    w3T = chunks(load_one("w3T", [128, 4, 512]), 4)
    oWT_t = load_one("oWT", [128, 8])              # col k = oWT chunk k
    oWT = [oWT_t[:, k:k + 1] for k in range(8)]

    # all small fc/conv biases packed into one [128, 19] f32 tensor
    biasP = load_one("biasP", [128, 19], F32)
    b_ib, b_pb, b_hb = biasP[:, 0:4], biasP[:, 4:6], biasP[:, 6:8]
    b_cb, b_c1, b_c2, b_c3 = biasP[:, 8:12], biasP[:, 12:13], \
        biasP[:, 13:15], biasP[:, 15:19]

    # ---------------- activations (att/iWT feed mid-GRU fillers; cWT is
    # only needed post-GRU)
    posT = ap_.tile([51, TB], BF16, tag="posT")
    nc.sync.dma_start(posT[:], io["posT"][:])
    probs = ap_.tile([128, NCH, K], F32, tag="probs")
    nc.sync.dma_start(probs[:], io["probsP"].rearrange("(c p) k -> p c k", p=128))
    sent = ap_.tile([128, E], BF16, tag="sent")   # pre-broadcast rows p -> b=p%16
    nc.sync.dma_start(sent[:], io["sentB"][:])
    hid = chunks(load_one("hidT", [128, 4, TB], pool=ap_), 4)
    att = chunks(load_one("attT", [128, 16, TB], pool=ap_), 16)
    iWT = chunks(load_one("iWT", [128, 16, 512]), 16)
    cWT = chunks(load_one("cWT", [128, 16, 512]), 16)

    def _dbg_out(src, n=TB):
        lgd = fp.tile([1, TB], F32, tag="lg", name="lgdbg")
        nc.vector.memset(lgd[:], 0.0)
        nc.vector.tensor_copy(lgd[0:1, 0:n], src)
        nc.sync.dma_start(io["out_logits"][:], lgd[:])

    if stage == 0:   # loads only
        _dbg_out(att[0][0:1, 0:TB])
        return

    # ---------------- capeT via PE transpose (no DMA transposes)
    capeT = gp.tile([128, 4, TBP2], BF16, tag="capeT", name="capeT")
    for c in range(NCH2):
        for half in range(2):
            tp = ps_.tile([128, 2, 128], BF16, tag="tp", name="tp_cape")
            for kk in (2 * half, 2 * half + 1):
                nc.tensor.transpose(
                    tp[:, kk - 2 * half, :],
                    capeall[:, c, kk * 128:(kk + 1) * 128], ident[:])
            nc.vector.tensor_copy(
                capeT[:, 2 * half:2 * half + 2, c * 128:(c + 1) * 128],
                tp[:])

    if stage == 1:   # gathers + capeT
        _dbg_out(capeT[0:1, 0, 0:TB])
        return

    # ---------------- gi = cap_e @ w_ih.T + (b_ih + b_hh) -> gx tiles
    # c=0 runs inline (feeds GRU steps 0-3); c=1..5 drain as early PE
    # fillers inside the GRU loop (forced before the step that reads them).
    pe_fill = Filler()
    gx = [None] * NCH2

    def gi_chunk(c, tags):
        def f():
            t = ap_.tile([128, G3], BF16, tag=f"gx_{c}", name=f"gx_{c}")
            for nb in range(3):
                ps = ps_.tile([128, 512], F32, tag=tags[nb % len(tags)],
                              name="gi_ps")
                sl = slice(nb * 512, (nb + 1) * 512)
                nc.tensor.matmul(ps[:], ones1[0:1, :], gbrow[0:1, sl],
                                 start=True, stop=False)
                for kk in range(4):
                    nc.tensor.matmul(
                        ps[:], capeT[:, kk, c * 128:(c + 1) * 128],
                        wih[kk][:, sl], start=False, stop=(kk == 3))
                nc.scalar.copy(t[:, sl], ps[:])
            gx[c] = t
        return f

    gi_chunk(0, ("gr", "gz", "gn"))()
    for c in range(1, NCH2):
        pe_fill.add(3200, gi_chunk(c, ("mm", "ctxp")))

    if stage == 2:   # + gi
        _dbg_out(gx[0][0:1, 0:TB])
        return

    # ---------------- uncertainty-feature dot products (DVE fillers)
    NP = len(PAIRS)
    Nt = fp.tile([128, NCH, K], F32, tag="norms")
    Gt = fp.tile([128, NCH, NP], F32, tag="gram")
    St = fp.tile([128, NCH, K], F32, tag="sdot")
    Ns = fp.tile([128, 1], F32, tag="snorm")

    def ttr(in0, in1, acc, eng):
        def f():
            scr = sp.tile([128, E], BF16, tag=f"ttr_scr{eng is nc.gpsimd}",
                          name="ttr_scr")
            eng.scalar_tensor_tensor(
                out=scr[:], in0=in0, scalar=1.0, in1=in1,
                op0=OP.mult, op1=OP.mult, accum_out=acc)
        return f

    def ttr_block(c, eng):
        items = []
        for k in range(K):
            items.append(ttr(emb[k][:, c, :], emb[k][:, c, :],
                             Nt[:, c, k:k + 1], eng))
        for i, (k, j) in enumerate(PAIRS):
            items.append(ttr(emb[k][:, c, :], emb[j][:, c, :],
                             Gt[:, c, i:i + 1], eng))
        for k in range(K):
            items.append(ttr(emb[k][:, c, :], sent[:], St[:, c, k:k + 1], eng))
        return items

    # DVE is idle while gi streams on the PE: run c=0 inline (they wait on
    # the emball gather); c=1,2 drain in-loop on DVE.
    ttr(sent[:], sent[:], Ns[:], nc.vector)()
    for fn in ttr_block(0, nc.vector):
        fn()
    dve_fill = Filler()
    for fn in ttr_block(1, nc.vector) + ttr_block(2, nc.vector):
        dve_fill.add(600, fn)

    # ---------------- PE fillers: pose/posf, imgf, hidf
    def mm_epilogue(ps, bias_tile, bias_col, tag, n=TB):
        t = fp.tile([128, n], BF16, tag=tag, name=tag)
        nc.scalar.activation(t[:], ps, AF.Prelu,
                             bias=bias_tile[:, bias_col:bias_col + 1], alpha=0.25)
        return t

    posf = [None, None]
    hidf = [None, None]
    imgf = [None, None, None, None]

    def do_pos():
        pps = ps_.tile([128, TB], F32, tag="ctxp", name="pose_ps")
        nc.tensor.matmul(pps[:], posW[:], posT[:], start=True, stop=True)
        pose = fp.tile([128, TB], BF16, tag="pose")
        nc.scalar.copy(pose[:], pps[:])
        for mc in range(2):
            ps = ps_.tile([128, TB], F32, tag="mm", name="posf_ps")
            nc.tensor.matmul(ps[:], pWT[:, mc * 128:(mc + 1) * 128], pose[:],
                             start=True, stop=True)
            posf[mc] = mm_epilogue(ps[:], b_pb, mc, f"posf{mc}")

    def do_hid():
        for mc in range(2):
            ps = ps_.tile([128, TB], F32, tag="mm", name="hidf_ps")
            for kk in range(4):
                nc.tensor.matmul(ps[:], hWT[kk][:, mc * 128:(mc + 1) * 128],
                                 hid[kk][:], start=(kk == 0), stop=(kk == 3))
            hidf[mc] = mm_epilogue(ps[:], b_hb, mc, f"hidf{mc}")

    def do_img(mc):
        def f():
            ps = ps_.tile([128, TB], F32, tag="mm", name="imgf_ps")
            for kk in range(16):
                nc.tensor.matmul(ps[:], iWT[kk][:, mc * 128:(mc + 1) * 128],
                                 att[kk][:], start=(kk == 0), stop=(kk == 15))
            imgf[mc] = mm_epilogue(ps[:], b_ib, mc, f"imgf{mc}")
        return f

    pe_fill.add(1800, do_pos)
    pe_fill.add(2400, do_hid)
    for mc in range(4):
        pe_fill.add(3900, do_img(mc))

    # ---------------- GRU (one direction per core, 32 pair-batch rows)
    h = ap_.tile([PB, H], BF16, tag="h")
    nc.vector.memset(h[:], 0.0)
    hTs = ap_.tile([128, 4, PB], BF16, tag="hTs")

    def sel_mms(t_, tags_slices):
        """Selector-matmuls moving gx rows (bias already folded into gx)
        for step t_ into fresh psum tiles. Returns dict tag->psum tile."""
        c, off = divmod(PB * t_, 128)
        out = {}
        for tag, gsl, stop in tags_slices:
            ps = ps_.tile([128, 512], F32, tag=tag, name=f"ps_{tag}")
            nc.tensor.matmul(ps[0:PB, :], ident[:, off:off + PB],
                             gx[c][:, gsl], start=True, stop=stop)
            out[tag] = ps
        return out

    cur = sel_mms(0, (("gr", SL_R, False), ("gz", SL_Z, False),
                      ("gx", SL_N, True)))

    for t_ in range(T):
        psr, psz, psx = cur["gr"], cur["gz"], cur["gx"]
        psn = None
        if t_ > 0:
            psn = ps_.tile([128, 512], F32, tag="gn", name="ps_gn")
            for gi_, (ps, gsl) in enumerate(
                    ((psr, SL_R), (psn, SL_N), (psz, SL_Z))):
                for kk in range(4):
                    nc.tensor.matmul(
                        ps[0:PB, :], hTs[:, kk, :], whh[kk][:, gsl],
                        start=(gi_ == 1 and kk == 0), stop=(kk == 3))
        # gate math; scratch per step
        rz = sp.tile([PB, H2], BF16, tag="rz", name="rz")
        n_ = sp.tile([PB, H], BF16, tag="n_", name="n_")
        hmn = sp.tile([PB, H], BF16, tag="hmn", name="hmn")
        # ACT FIFO order r-sig, z-sig, tanh: z-sig must not queue behind the
        # npre-stalled tanh ops (the h update needs both tanh AND z-sig)
        nc.scalar.activation(rz[:, SL_R], psr[0:PB, :], AF.Sigmoid)
        if t_ > 0:
            npre = sp.tile([PB, H], BF16, tag="npre", name="npre")
            nc.vector.tensor_tensor(npre[:], psn[0:PB, :], rz[:, SL_R], OP.mult)
            nc.vector.tensor_tensor(npre[:], npre[:], psx[0:PB, :], OP.add)
            tanh_src = npre
        else:
            tanh_src = psx[0:PB, :]
        nc.scalar.activation(rz[:, SL_Z], psz[0:PB, :], AF.Sigmoid)
        for hh in range(2):
            sl = slice(hh * 256, (hh + 1) * 256)
            nc.scalar.activation(n_[:, sl], tanh_src[:, sl], AF.Tanh)
        # h update chunked by 128 cols; PE transpose each chunk; copies
        # alternate ACT/DVE so the first chunk unblocks next step's matmuls
        tpA = ps_.tile([128, 2, PB], BF16, tag="tp", name="tpA")
        tpB = ps_.tile([128, 2, PB], BF16, tag="tp", name="tpB")
        for half in range(2):
            cs = slice(half * 256, (half + 1) * 256)
            zs = slice(H + half * 256, H + (half + 1) * 256)
            nc.vector.tensor_tensor(hmn[:, cs], h[:, cs], n_[:, cs], OP.subtract)
            nc.vector.tensor_tensor(hmn[:, cs], hmn[:, cs], rz[:, zs], OP.mult)
            nc.vector.tensor_tensor(h[:, cs], n_[:, cs], hmn[:, cs], OP.add)
            tp = (tpA, tpB)[half]
            for ci in range(2):
                cc = 2 * half + ci
                nc.tensor.transpose(tp[:, ci, :],
                                    h[0:PB, cc * 128:(cc + 1) * 128],
                                    ident[0:PB, 0:PB])
            if half == 0:
                nc.scalar.copy(hTs[:, 0:2, :], tp[:])
            else:
                nc.vector.tensor_copy(hTs[:, 2:4, :], tp[:])
        if t_ + 1 < T:
            cnx = (PB * (t_ + 1)) // 128
            while gx[cnx] is None:
                pe_fill.step(3200)   # force-drain until the gx chunk exists
            cur = sel_mms(t_ + 1, (("gr", SL_R, False), ("gz", SL_Z, False),
                                   ("gx", SL_N, True)))
        # fillers into this step's engine gaps (front-loaded: the gi c>=1
        # items must land before step 4c reads their gx chunks)
        if t_ >= 1:
            pe_fill.step(1700 if t_ <= 10 else 1200)
            dve_fill.step(1000)

    # ---------------- pairwise h_f/h_b exchange (issued before uf/CNN so
    # the collective hides behind h-independent PE work)
    hout = dr.tile([PB, E], BF16, tag="hout", name="hout")
    hgat = dr.tile([2 * PB, E], BF16, tag="hgat", name="hgat")
    nc.gpsimd.dma_start(hout[:], h[:])
    nc.gpsimd.collective_compute(
        "AllGather", OP.bypass,
        replica_groups=[[0, 4], [1, 5], [2, 6], [3, 7]],
        ins=[hout.opt()], outs=[hgat.opt()])
    hfin = ap_.tile([PB, E], BF16, tag="hfin")   # rows 0:16 h_f, 16:32 h_b
    nc.gpsimd.indirect_dma_start(
        out=hfin[:], out_offset=None, in_=hgat[:],
        in_offset=bass.IndirectOffsetOnAxis(ap=gidx[0:PB, 24:25], axis=0))

    pe_fill.flush()
    dve_fill.flush()

    if stage == 6:   # + GRU
        _dbg_out(h[0:1, 0:TB])
        return

    CT = ("mm", "ctxp")
    # ---------------- uncertainty features -> uf
    # all sqrt inputs packed into one tile so ONE Sqrt ACT op serves md,
    # cprod and sprod (single sqrt-set table residency window)
    uf = fp.tile([128, NCH, 4 * K], BF16, tag="uf")  # feature row = k*4 + ci
    rp = fp.tile([128, NCH, 17], F32, tag="rootpack")  # 0:6 md, 6:11 cp, 11:17 sp
    rq = fp.tile([128, NCH, 17], F32, tag="rootout")

    # min_dist: d2(k,j) = n_k + n_j - 2 g_kj; min over partners; sqrt
    npair = fp.tile([128, NCH, NP], F32, tag="npair")
    for i, (k, j) in enumerate(PAIRS):
        nc.vector.tensor_tensor(npair[:, :, i], Nt[:, :, k], Nt[:, :, j], OP.add)
    d2 = fp.tile([128, NCH, NP], F32, tag="d2")
    nc.vector.scalar_tensor_tensor(out=d2[:], in0=Gt[:], scalar=-2.0,
                                   in1=npair[:], op0=OP.mult, op1=OP.add)
    pidx = {}
    for i, (k, j) in enumerate(PAIRS):
        pidx[(k, j)] = i
        pidx[(j, k)] = i
    md = rp[:, :, 0:K]
    for k in range(K):
        parts = [pidx[(k, j)] for j in range(K) if j != k]
        nc.vector.tensor_tensor(md[:, :, k], d2[:, :, parts[0]],
                                d2[:, :, parts[1]], OP.min)
        for i in parts[2:]:
            nc.vector.tensor_tensor(md[:, :, k], md[:, :, k], d2[:, :, i], OP.min)
    nc.vector.tensor_scalar_max(md, md, 0.0)

    # cos_dist (k>0) needs rsqrt(n_0 * n_k); sentence needs rsqrt(ns * n_k)
    cprod = rp[:, :, K:K + 5]
    for c in range(NCH):
        nc.vector.tensor_scalar(out=cprod[:, c, :], in0=Nt[:, c, 1:],
                                scalar1=Nt[:, c, 0:1], scalar2=None, op0=OP.mult)
    sprod = rp[:, :, K + 5:K + 11]
    nc.vector.tensor_scalar(out=sprod, in0=Nt[:], scalar1=Ns[:, 0:1],
                            scalar2=None, op0=OP.mult)
    nc.vector.reciprocal(rp[:, :, K:K + 11], rp[:, :, K:K + 11])
    nc.scalar.activation(rq[:], rp[:], AF.Sqrt)   # the only sqrt-set op

    nc.vector.tensor_copy(uf[:, :, 0::4], rq[:, :, 0:K])
    nc.vector.tensor_tensor(uf[:, :, 5:24:4], Gt[:, :, 0:K - 1],
                            rq[:, :, K:K + 5], OP.mult)
    nc.vector.memset(uf[:, :, 1], 0.0)
    nc.vector.tensor_tensor(uf[:, :, 2::4], St[:], rq[:, :, K + 5:K + 11], OP.mult)
    nc.vector.tensor_copy(uf[:, :, 3::4], probs[:])

    if stage == 3:   # + uncertainty features
        _dbg_out(uf[0:1, :, :].rearrange("p c k -> p (c k)"), n=NCH * 24)
        return

    # ---------------- conv windows via PE transpose, then the CNN
    win = []
    for l in range(4):
        wps = ps_.tile([12, TBP], BF16, tag="tp", name="win_ps")
        for c in range(NCH):
            nc.tensor.transpose(wps[0:12, c * 128:(c + 1) * 128],
                                uf[:, c, 4 * l:4 * l + 12], ident[:])
        t = fp.tile([12, TBP], BF16, tag=f"win{l}", name=f"win{l}")
        nc.vector.tensor_copy(t[:], wps[:])
        win.append(t)

    c1 = []
    for l in range(4):
        ps = ps_.tile([128, TBP], F32, tag=CT[l % 2], name="c1_ps")
        nc.tensor.matmul(ps[:], w1T[:], win[l][:], start=True, stop=True)
        c1.append(mm_epilogue(ps[:], b_c1, 0, f"c1_{l}", n=TBP))
    c2 = []
    for lp in range(2):
        for mc in range(2):
            ps = ps_.tile([128, TBP], F32, tag=CT[mc % 2], name="c2_ps")
            for dk in range(3):
                nc.tensor.matmul(ps[:], w2T[dk][:, mc * 128:(mc + 1) * 128],
                                 c1[lp + dk][:], start=(dk == 0), stop=(dk == 2))
            c2.append(mm_epilogue(ps[:], b_c2, mc, f"c2_{lp}{mc}", n=TBP))
    unc = []
    for mc in range(4):
        ps = ps_.tile([128, TBP], F32, tag=CT[mc % 2], name="c3_ps")
        for kk in range(4):
            nc.tensor.matmul(ps[:], w3T[kk][:, mc * 128:(mc + 1) * 128],
                             c2[kk][:], start=(kk == 0), stop=(kk == 3))
        unc.append(mm_epilogue(ps[:], b_c3, mc, f"unc{mc}", n=TBP))

    if stage == 4:   # + CNN
        _dbg_out(unc[0][0:1, 0:TB])
        return

    # ---------------- final hT (own 16 rows each direction) for ctx
    hTfin = ap_.tile([128, 4, PB], BF16, tag="hTfin")  # cols 0:16 f, 16:32 b
    tpH = ps_.tile([128, 4, PB], BF16, tag="tp", name="tpH")
    for cc in range(4):
        nc.tensor.transpose(tpH[:, cc, :], hfin[0:PB, cc * 128:(cc + 1) * 128],
                            ident[0:PB, 0:PB])
    nc.vector.tensor_copy(hTfin[:], tpH[:])

    # ---------------- ctx = prelu(concat @ cW.T + cb), then logits
    # rhs chunk order matches cW rows: cap_f(4) cap_b(4) pos(2) img(4) hid(2)
    rhs_ctx = []
    for di in range(2):
        for cc in range(4):
            rhs_ctx.append(hTfin[:, cc, 16 * di:16 * di + 16].unsqueeze(1)
                           .broadcast_to([128, T, BPC]))
    rhs_ctx += [t[:] for t in posf + imgf + hidf]
    ctxa = []
    for mc in range(4):
        cps = ps_.tile([128, TB], F32, tag=CT[mc % 2], name="ctx_ps")
        for kk in range(16):
            nc.tensor.matmul(cps[:], cWT[kk][:, mc * 128:(mc + 1) * 128],
                             rhs_ctx[kk], start=(kk == 0), stop=(kk == 15))
        ctxa.append(mm_epilogue(cps[:], b_cb, mc, f"ctxa{mc}"))

    psl = ps_.tile([1, TB], F32, tag="mm", name="lg_ps")
    rhs_o = [t[:] for t in ctxa] + [t[:, 0:TB] for t in unc]
    for kk in range(8):
        nc.tensor.matmul(psl[:], oWT[kk][:], rhs_o[kk],
                         start=(kk == 0), stop=(kk == 7))
    lg = fp.tile([1, TB], F32, tag="lg")
    nc.scalar.copy(lg[:], psl[:])
    nc.sync.dma_start(io["out_logits"][:], lg[:])


# ---------------------------------------------------------------- build

@functools.lru_cache(maxsize=8)
def _build(stage=99):
    nc = bacc.Bacc("TRN2", target_bir_lowering=False, debug=False,
                   enable_asserts=False, num_devices=NCORE)
    io = {}

    def din(name, shape, dt):
        io[name] = nc.dram_tensor(name, list(shape), dt, kind="ExternalInput").ap()

    din("attT", [128, 16 * TB], BF16)
    din("hidT", [128, 4 * TB], BF16)
    din("posT", [51, TB], BF16)
    din("probsP", [TBP, K], F32)
    din("sentB", [128, E], BF16)
    din("ident", [128, 128], BF16)
    din("smallp", [1, G3 + 128], BF16)
    din("cap_emb_bf", [CV, E], BF16)
    din("cap_emb_w_bf", [CV, E], BF16)
    din("gidx", [128, 25], mybir.dt.int32)
    din("iWT", [128, 16 * 512], BF16)
    din("cWT", [128, 16 * 512], BF16)
    din("hWT", [128, 4 * 256], BF16)
    din("pWT", [128, 256], BF16)
    din("posW", [51, 128], BF16)
    din("w1T", [12, 128], BF16)
    din("w2T", [128, 3 * 256], BF16)
    din("w3T", [128, 4 * 512], BF16)
    din("oWT", [128, 8], BF16)
    din("wihT", [128, 4 * G3], BF16)
    din("whhT", [128, 4 * G3], BF16)
    din("biasP", [128, 19], F32)
    io["out_logits"] = nc.dram_tensor("out_logits", [1, TB], F32,
                                      kind="ExternalOutput").ap()

    with tile.TileContext(nc) as tc:
        _emit(nc, tc, io, stage)
    nc.compile()
    return nc


# ---------------------------------------------------------------- host side

def _bf(x):
    return np.ascontiguousarray(np.asarray(x, np.float32).astype(BF))


def _pmaj(a, k):
    """[k*128, n] -> [128, k*n] partition-major interleave (one-DMA layout)."""
    a = np.asarray(a)
    return np.ascontiguousarray(
        a.reshape(k, 128, -1).transpose(1, 0, 2).reshape(128, -1))


def _prep_core(ci, inp, shared):
    sl = slice(ci * BPC, (ci + 1) * BPC)
    attT = _pmaj(_bf(np.asarray(inp["attended_img"])[sl]
                     .transpose(2, 1, 0).reshape(IC, TB)), 16)
    hidT = _pmaj(_bf(np.asarray(inp["hidden"])[sl]
                     .transpose(2, 1, 0).reshape(512, TB)), 4)
    posT = _bf(np.asarray(inp["pos"])[sl].transpose(2, 1, 0).reshape(51, TB))
    probsP = np.zeros((TBP, K), np.float32)
    probsP[:TB] = np.asarray(inp["topk_probs"])[:, sl, :].reshape(TB, K)
    tw = np.asarray(inp["topk_words"])[:, sl, :].astype(np.int64)  # [21, 16, 6]

    def pad(a, n):
        o = np.zeros(n, np.int64)
        o[:a.shape[0]] = a
        return o

    gidx = np.zeros((128, 25), np.int32)
    for k in range(K):
        col = pad(tw[:, :, k].reshape(TB), TBP)
        for c in range(NCH):
            gidx[:, k * NCH + c] = col[c * 128:(c + 1) * 128]

    # GRU pair: caption order for this core's direction over 32 pair rows
    lo = ci % 4
    caption = np.asarray(inp["caption"]).astype(np.int64)
    cap2 = np.concatenate([caption[lo * BPC:(lo + 1) * BPC],
                           caption[(lo + 4) * BPC:(lo + 5) * BPC]], axis=0)
    order = cap2.T.reshape(TB2) if ci < 4 else cap2[:, ::-1].T.reshape(TB2)
    col = pad(order, TBP2)
    for c in range(NCH2):
        gidx[:, 18 + c] = col[c * 128:(c + 1) * 128]
    # h-exchange row select: my own 16 rows of [f(32) | b(32)] gathered order
    p = 0 if ci < 4 else 1
    gidx[0:BPC, 24] = np.arange(BPC) + BPC * p
    gidx[BPC:PB, 24] = np.arange(BPC) + PB + BPC * p

    d = "f" if ci < 4 else "b"
    m = {
        "attT": attT, "hidT": hidT, "posT": posT, "probsP": probsP,
        "sentB": np.ascontiguousarray(np.tile(shared["sent"][sl], (8, 1))),
        "gidx": gidx,
        "wihT": shared["weights"][f"wihT_{d}"],
        "whhT": shared["weights"][f"whhT_{d}"],
        "smallp": shared["weights"][f"smallp_{d}"],
    }
    for k, v in shared["weights"].items():
        if not (k.startswith("wihT_") or k.startswith("whhT_")
                or k.startswith("smallp_")):
            m[k] = v
    return m


def _prep_shared(inp):
    cap_emb = np.asarray(inp["cap_embedding"], np.float32)
    capt = np.asarray(inp["caption"]).astype(np.int64)
    cap_len = np.asarray(inp["cap_len"]).astype(np.int64)
    mask = (np.arange(T)[None, :] < cap_len[:, None]).astype(np.float32)
    sent = np.einsum("bte,bt->be", cap_emb[capt], mask)       # [B, E]

    w = {}
    w["cap_emb_bf"] = _bf(cap_emb)
    w["cap_emb_w_bf"] = _bf(inp["cap_emb_w"])
    w["iWT"] = _pmaj(_bf(np.asarray(inp["iW"], np.float32).T), 16)
    w["cWT"] = _pmaj(_bf(np.asarray(inp["cW"], np.float32).T), 16)
    w["hWT"] = _pmaj(_bf(np.asarray(inp["hW"], np.float32).T), 4)
    w["pWT"] = _bf(np.asarray(inp["pW"], np.float32).T)
    w["posW"] = _bf(inp["pos_emb_w"])
    w["w1T"] = _bf(np.asarray(inp["conv1_w"], np.float32).transpose(2, 1, 0).reshape(12, 128))
    w["w2T"] = _pmaj(_bf(np.asarray(inp["conv2_w"], np.float32).transpose(2, 1, 0).reshape(384, 256)), 3)
    w["w3T"] = _pmaj(_bf(np.asarray(inp["conv3_w"], np.float32).transpose(2, 1, 0).reshape(512, 512)), 4)
    w["oWT"] = np.ascontiguousarray(
        _bf(np.asarray(inp["oW"], np.float32).T).reshape(8, 128).T)
    ones = np.ones((128,), np.float32)
    for d, sfx in (("f", "_f"), ("b", "_b")):
        w[f"wihT_{d}"] = _pmaj(_bf(np.asarray(inp["gru_w_ih" + sfx], np.float32).T), 4)
        w[f"whhT_{d}"] = _pmaj(_bf(np.asarray(inp["gru_w_hh" + sfx], np.float32).T), 4)
        bias = np.asarray(inp["gru_b_ih" + sfx], np.float32) \
            + np.asarray(inp["gru_b_hh" + sfx], np.float32)
        w[f"smallp_{d}"] = _bf(np.concatenate([ones, bias]).reshape(1, -1))
    bias_cols = []
    for key in ("ib", "pb", "hb", "cb", "conv1_b", "conv2_b", "conv3_b"):
        bias_cols.append(np.asarray(inp[key], np.float32).reshape(-1, 128).T)
    w["biasP"] = np.ascontiguousarray(np.concatenate(bias_cols, axis=1))
    w["ident"] = _bf(np.eye(128, dtype=np.float32))
    return {"weights": w, "sent": _bf(sent).astype(BF)}


_RUNNER = {}


def _get_runner():
    if "fn" not in _RUNNER:
        nc = _build()
        _RUNNER["nc"] = nc
        _RUNNER["fn"] = lambda in_maps: run_bass_kernel_spmd(
            nc, in_maps, core_ids=list(range(NCORE)))
    return _RUNNER["fn"]


def kernel(**inputs):
    fn = _get_runner()
    shared = _prep_shared(inputs)
    in_maps = [_prep_core(ci, inputs, shared) for ci in range(NCORE)]
    res = fn(in_maps)
    logits = np.zeros((B, T), np.float32)
    for ci in range(NCORE):
        lg = np.asarray(res.results[ci]["out_logits"], np.float32).reshape(TB)
        logits[ci * BPC:(ci + 1) * BPC] = lg.reshape(T, BPC).T
    logits += float(np.asarray(inputs["ob"]).reshape(-1)[0])
    pos = np.asarray(inputs["pos"])
    valid_pos = np.argmax(pos, axis=2) != (pos.shape[-1] - 2)
    return logits, valid_pos


# revision 5
# speedup vs baseline: 1.2108x; 1.2108x over previous
"""Trainium2 Bass kernel for nn_DecisionMaker (retrieval_knn), v4.

v4 = v3 (direction-split GRU across core pairs) + latency work:
 - all embedding gathers are host-prepared direct inputs (capeT, embP);
   the kernel has no SWDGE indirect DMAs left. This removes the slow
   gather+transpose preamble in front of gi.
 - the GRU gate math runs in 256-col halves with first-half priority and
   per-128-chunk hTs copies, so the next step's whh matmuls start ~1.5us
   earlier; next-step selectors are emitted right after this step's whh
   so they execute inside the gate-math window; the n-gate selector psum
   alternates banks (gx0/gx1) to kill the write-after-read stall.
 - the pairwise AllGather result is pulled into SBUF with one DMA and
   the core's own h_f/h_b rows are selected with a single 0/1-matrix
   matmul (hselM input) instead of a 32-descriptor SWDGE gather.
 - ctx accumulates its pos/img/hid chunks (and the CNN, held-back imgf
   and hidf fillers run) while the collective is in flight; the cap
   chunks are accumulated last. The logits matmul accumulates unc first
   (oWT host-packed in [unc|ctx] order).

Data-parallel layout as before: per-core own rows tb = t*16+b (TB=336);
GRU pair rows tb2 = t*32+b2 (TB2=672, 6 chunks of 128).
"""

import functools
from collections import deque

import numpy as np
import ml_dtypes

import concourse.bass as bass
import concourse.tile as tile
from concourse import bacc, mybir
from concourse.bass_utils import run_bass_kernel_spmd

F32 = mybir.dt.float32
BF16 = mybir.dt.bfloat16
AF = mybir.ActivationFunctionType
OP = mybir.AluOpType
BF = ml_dtypes.bfloat16

NCORE = 8
B, T, K, E, H, IC = 128, 21, 6, 512, 512, 2048
CV = 10001
BPC = B // NCORE            # 16 batch rows per core (data-parallel parts)
PB = 2 * BPC                # 32 batch rows per GRU pair
TB = T * BPC                # 336 (t-major: tb = t*16 + b)
TBP = 384                   # padded to 3*128
NCH = 3                     # tb chunks of 128
TB2 = T * PB                # 672 (GRU: tb2 = t*32 + b2)
NCH2 = 6                    # tb2 chunks of 128 (672 -> 768 padded)
TBP2 = NCH2 * 128
G3 = 3 * H                  # 1536 gate width
H2 = 2 * H
PAIRS = [(i, j) for i in range(K) for j in range(i + 1, K)]  # 15, (0,k) first
# gate column slices in the PyTorch order r, z, n
SL_R = slice(0, H)
SL_Z = slice(H, H2)
SL_N = slice(H2, G3)


class Filler:
    """Credit-based queue of (cost_ns, emit_fn) items drained into gaps."""

    def __init__(self):
        self.q = deque()
        self.credit = 0.0

    def add(self, cost, fn):
        self.q.append((cost, fn))

    def step(self, budget):
        self.credit += budget
        while self.q and self.q[0][0] <= self.credit:
            cost, fn = self.q.popleft()
            fn()
            self.credit -= cost

    def flush(self):
        while self.q:
            self.q.popleft()[1]()
        self.credit = 0.0


def _emit(nc, tc, io, stage=99):
    from contextlib import ExitStack
    ctx = ExitStack()
    wp = ctx.enter_context(tc.tile_pool(name="weights", bufs=1))
    ap_ = ctx.enter_context(tc.tile_pool(name="acts", bufs=1))
    sp = ctx.enter_context(tc.tile_pool(name="scratch", bufs=2))
    fp = ctx.enter_context(tc.tile_pool(name="feats", bufs=1))
    ps_ = ctx.enter_context(tc.tile_pool(name="psum", bufs=1, space="PSUM"))
    dr = ctx.enter_context(tc.tile_pool(name="dram", bufs=1, space="DRAM"))
    # stage 98 = 8x full body; 97 = 16x; 96 = 24x; 900+S = 8x stage-S body
    # experimental body: 72 = no collective (local DMA stand-in)
    nrep, body = {98: (8, 99), 97: (16, 99), 96: (24, 99)}.get(
        stage, (8, stage - 900) if stage >= 900 else (1, stage))
    for _rep in range(nrep):
        _emit_body(nc, tc, io, body, ctx, wp, ap_, sp, fp, ps_, dr)
    ctx.close()


def _emit_body(nc, tc, io, stage, ctx, wp, ap_, sp, fp, ps_, dr):

    # ---------------- resident weights/activations (bf16, host pre-arranged
    # so every multi-chunk tensor loads with ONE contiguous DMA)
    def load_one(name, shape, dt=BF16, pool=None):
        t = (pool or wp).tile(list(shape), dt, tag=name, name=name)
        nc.sync.dma_start(t[:], io[name][:].rearrange(
            "p (k n) -> p k n", k=shape[1]) if len(shape) == 3 else io[name][:])
        return t

    def chunks(t, n):
        return [t[:, k, :] for k in range(n)]

    # critical-path order: capeT/wih feed gi -> GRU
    capeT = load_one("capeT", [128, 4, TBP2], pool=ap_)
    ident = load_one("ident", [128, 128])          # bf16 identity
    smallp = load_one("smallp", [1, G3 + 128])     # ones row + gru bias (own dir)
    ones1 = smallp[0:1, 0:128]
    gbrow = smallp[0:1, 128:128 + G3]
    wih = chunks(load_one("wihT", [128, 4, G3]), 4)
    whh = chunks(load_one("whhT", [128, 4, G3]), 4)
    hsel = load_one("hselM", [2 * PB, PB])
    posW = load_one("posW", [51, 128])             # pos_emb_w, natural lhsT
    w1T = load_one("w1T", [12, 128])
    pWT = load_one("pWT", [128, 256])
    hWT = chunks(load_one("hWT", [128, 4, 256]), 4)
    w2T = chunks(load_one("w2T", [128, 3, 256]), 3)
    w3T = chunks(load_one("w3T", [128, 4, 512]), 4)
    oWT_t = load_one("oWT", [128, 8])              # col k: [unc(4) | ctx(4)]
    oWT = [oWT_t[:, k:k + 1] for k in range(8)]

    # all small fc/conv biases packed into one [128, 19] f32 tensor
    biasP = load_one("biasP", [128, 19], F32)
    b_ib, b_pb, b_hb = biasP[:, 0:4], biasP[:, 4:6], biasP[:, 6:8]
    b_cb, b_c1, b_c2, b_c3 = biasP[:, 8:12], biasP[:, 12:13], \
        biasP[:, 13:15], biasP[:, 15:19]

    embP = load_one("embP", [128, K * NCH, E], pool=ap_)
    emb = [embP[:, k * NCH:(k + 1) * NCH, :] for k in range(K)]
    posT = ap_.tile([51, TB], BF16, tag="posT")
    nc.sync.dma_start(posT[:], io["posT"][:])
    probs = ap_.tile([128, NCH, K], F32, tag="probs")
    nc.sync.dma_start(probs[:], io["probsP"].rearrange("(c p) k -> p c k", p=128))
    sent = ap_.tile([128, E], BF16, tag="sent")   # pre-broadcast rows p -> b=p%16
    nc.sync.dma_start(sent[:], io["sentB"][:])
    hid = chunks(load_one("hidT", [128, 4, TB], pool=ap_), 4)
    att = chunks(load_one("attT", [128, 16, TB], pool=ap_), 16)
    iWT = chunks(load_one("iWT", [128, 16, 512]), 16)
    cWT = chunks(load_one("cWT", [128, 16, 512]), 16)

    def _dbg_out(src, n=TB):
        lgd = fp.tile([1, TB], F32, tag="lg", name="lgdbg")
        nc.vector.memset(lgd[:], 0.0)
        nc.vector.tensor_copy(lgd[0:1, 0:n], src)
        nc.sync.dma_start(io["out_logits"][:], lgd[:])

    if stage == 0:   # loads only
        _dbg_out(att[0][0:1, 0:TB])
        return

    # ---------------- gi = cap_e @ w_ih.T + (b_ih + b_hh) -> gx tiles
    # c=0 runs inline (feeds GRU steps 0-3); c=1..5 drain as early PE
    # fillers inside the GRU loop (forced before the step that reads them).
    pe_fill = Filler()
    gx = [None] * NCH2

    def gi_chunk(c, tags):
        def f():
            t = ap_.tile([128, G3], BF16, tag=f"gx_{c}", name=f"gx_{c}")
            for nb in range(3):
                ps = ps_.tile([128, 512], F32, tag=tags[nb % len(tags)],
                              name="gi_ps")
                sl = slice(nb * 512, (nb + 1) * 512)
                nc.tensor.matmul(ps[:], ones1[0:1, :], gbrow[0:1, sl],
                                 start=True, stop=False)
                for kk in range(4):
                    nc.tensor.matmul(
                        ps[:], capeT[:, kk, c * 128:(c + 1) * 128],
                        wih[kk][:, sl], start=False, stop=(kk == 3))
                nc.scalar.copy(t[:, sl], ps[:])
            gx[c] = t
        return f

    gi_chunk(0, ("gr", "gz", "gn"))()
    for c in range(1, NCH2):
        pe_fill.add(3200, gi_chunk(c, ("mm", "ctxp")))

    if stage == 2:   # + gi
        _dbg_out(gx[0][0:1, 0:TB])
        return

    # ---------------- uncertainty-feature dot products (DVE fillers)
    NP = len(PAIRS)
    Nt = fp.tile([128, NCH, K], F32, tag="norms")
    Gt = fp.tile([128, NCH, NP], F32, tag="gram")
    St = fp.tile([128, NCH, K], F32, tag="sdot")
    Ns = fp.tile([128, 1], F32, tag="snorm")

    def ttr(in0, in1, acc, eng):
        def f():
            scr = sp.tile([128, E], BF16, tag=f"ttr_scr{eng is nc.gpsimd}",
                          name="ttr_scr")
            eng.scalar_tensor_tensor(
                out=scr[:], in0=in0, scalar=1.0, in1=in1,
                op0=OP.mult, op1=OP.mult, accum_out=acc)
        return f

    def ttr_block(c, eng):
        items = []
        for k in range(K):
            items.append(ttr(emb[k][:, c, :], emb[k][:, c, :],
                             Nt[:, c, k:k + 1], eng))
        for i, (k, j) in enumerate(PAIRS):
            items.append(ttr(emb[k][:, c, :], emb[j][:, c, :],
                             Gt[:, c, i:i + 1], eng))
        for k in range(K):
            items.append(ttr(emb[k][:, c, :], sent[:], St[:, c, k:k + 1], eng))
        return items

    ttr(sent[:], sent[:], Ns[:], nc.vector)()
    for fn in ttr_block(0, nc.vector):
        fn()
    dve_fill = Filler()
    for fn in ttr_block(1, nc.vector) + ttr_block(2, nc.vector):
        dve_fill.add(600, fn)

    # ---------------- PE fillers: pose/posf, imgf 0-1 (2-3 + hidf are held
    # back to cover the post-GRU collective window)
    def mm_epilogue(ps, bias_tile, bias_col, tag, n=TB):
        t = fp.tile([128, n], BF16, tag=tag, name=tag)
        nc.scalar.activation(t[:], ps, AF.Prelu,
                             bias=bias_tile[:, bias_col:bias_col + 1], alpha=0.25)
        return t

    posf = [None, None]
    hidf = [None, None]
    imgf = [None, None, None, None]

    def do_pos():
        pps = ps_.tile([128, TB], F32, tag="ctxp", name="pose_ps")
        nc.tensor.matmul(pps[:], posW[:], posT[:], start=True, stop=True)
        pose = fp.tile([128, TB], BF16, tag="pose")
        nc.scalar.copy(pose[:], pps[:])
        for mc in range(2):
            ps = ps_.tile([128, TB], F32, tag="mm", name="posf_ps")
            nc.tensor.matmul(ps[:], pWT[:, mc * 128:(mc + 1) * 128], pose[:],
                             start=True, stop=True)
            posf[mc] = mm_epilogue(ps[:], b_pb, mc, f"posf{mc}")

    def do_hid():
        for mc in range(2):
            ps = ps_.tile([128, TB], F32, tag="mm", name="hidf_ps")
            for kk in range(4):
                nc.tensor.matmul(ps[:], hWT[kk][:, mc * 128:(mc + 1) * 128],
                                 hid[kk][:], start=(kk == 0), stop=(kk == 3))
            hidf[mc] = mm_epilogue(ps[:], b_hb, mc, f"hidf{mc}")

    def do_img(mc):
        def f():
            ps = ps_.tile([128, TB], F32, tag="mm", name="imgf_ps")
            for kk in range(16):
                nc.tensor.matmul(ps[:], iWT[kk][:, mc * 128:(mc + 1) * 128],
                                 att[kk][:], start=(kk == 0), stop=(kk == 15))
            imgf[mc] = mm_epilogue(ps[:], b_ib, mc, f"imgf{mc}")
        return f

    pe_fill.add(1800, do_pos)
    for mc in range(2):
        pe_fill.add(3900, do_img(mc))

    # ---------------- GRU (one direction per core, 32 pair-batch rows)
    h = ap_.tile([PB, H], BF16, tag="h")
    nc.vector.memset(h[:], 0.0)
    hTs = ap_.tile([128, 4, PB], BF16, tag="hTs")
    GXT = ("gx0", "gx1")

    def sel_mms(t_):
        """Selector-matmuls moving gx rows (bias already folded into gx)
        for step t_ into fresh psum tiles. n-gate psum alternates banks."""
        c, off = divmod(PB * t_, 128)
        out = {}
        for tag, gsl, stop in (("gr", SL_R, False), ("gz", SL_Z, False),
                               (GXT[t_ % 2], SL_N, True)):
            ps = ps_.tile([128, 512], F32, tag=tag, name=f"ps_{tag}")
            nc.tensor.matmul(ps[0:PB, :], ident[:, off:off + PB],
                             gx[c][:, gsl], start=True, stop=stop)
            out[tag[:2]] = ps
        return out

    cur = sel_mms(0)

    for t_ in range(T):
        psr, psz, psx = cur["gr"], cur["gz"], cur["gx"]
        psn = None
        if t_ > 0:
            psn = ps_.tile([128, 512], F32, tag="gn", name="ps_gn")
            for gi_, (ps, gsl) in enumerate(
                    ((psr, SL_R), (psn, SL_N), (psz, SL_Z))):
                for kk in range(4):
                    nc.tensor.matmul(
                        ps[0:PB, :], hTs[:, kk, :], whh[kk][:, gsl],
                        start=(gi_ == 1 and kk == 0), stop=(kk == 3))
        # next step's selectors early: they run on PE inside this step's
        # gate-math window (fillers may slot in just before them)
        nxt = None
        if t_ + 1 < T:
            if t_ >= 1:
                pe_fill.step(1500)
                dve_fill.step(900)
            cnx = (PB * (t_ + 1)) // 128
            while gx[cnx] is None:
                pe_fill.step(3200)   # force-drain until the gx chunk exists
            nxt = sel_mms(t_ + 1)
        # gate math in 256-col halves, first-half priority, so the first
        # hTs chunks (and next step's whh) start as early as possible
        rz = sp.tile([PB, H2], BF16, tag="rz", name="rz")
        n_ = sp.tile([PB, H], BF16, tag="n_", name="n_")
        hmn = sp.tile([PB, H], BF16, tag="hmn", name="hmn")
        npre = sp.tile([PB, H], BF16, tag="npre", name="npre") if t_ > 0 else None
        tpA = ps_.tile([128, 2, PB], BF16, tag="tp", name="tpA")
        tpB = ps_.tile([128, 2, PB], BF16, tag="tp", name="tpB")
        for half in range(2):
            cs = slice(half * 256, (half + 1) * 256)
            zs = slice(H + half * 256, H + (half + 1) * 256)
            nc.scalar.activation(rz[:, cs], psr[0:PB, cs], AF.Sigmoid)
            if half == 0:   # z-sig early; h-update needs it after tanh
                nc.scalar.activation(rz[:, SL_Z], psz[0:PB, :], AF.Sigmoid)
            if t_ > 0:
                nc.vector.tensor_tensor(npre[:, cs], psn[0:PB, cs],
                                        rz[:, cs], OP.mult)
                nc.vector.tensor_tensor(npre[:, cs], npre[:, cs],
                                        psx[0:PB, cs], OP.add)
                tanh_src = npre[:, cs]
            else:
                tanh_src = psx[0:PB, cs]
            nc.scalar.activation(n_[:, cs], tanh_src, AF.Tanh)
            nc.vector.tensor_tensor(hmn[:, cs], h[:, cs], n_[:, cs], OP.subtract)
            nc.vector.tensor_tensor(hmn[:, cs], hmn[:, cs], rz[:, zs], OP.mult)
            nc.vector.tensor_tensor(h[:, cs], n_[:, cs], hmn[:, cs], OP.add)
            tp = (tpA, tpB)[half]
            for ci in range(2):
                cc = 2 * half + ci
                nc.tensor.transpose(tp[:, ci, :],
                                    h[0:PB, cc * 128:(cc + 1) * 128],
                                    ident[0:PB, 0:PB])
            # per-128-chunk copies: whh(t+1) kk=0 starts on the first chunk
            if half == 0:
                nc.scalar.copy(hTs[:, 0:1, :], tp[:, 0:1, :])
                nc.scalar.copy(hTs[:, 1:2, :], tp[:, 1:2, :])
            else:
                nc.vector.tensor_copy(hTs[:, 2:3, :], tp[:, 0:1, :])
                nc.vector.tensor_copy(hTs[:, 3:4, :], tp[:, 1:2, :])
        cur = nxt

    # ---------------- pairwise h_f/h_b exchange (issued immediately; the
    # CNN / held-back fillers / ctx noncap chunks run while it flies)
    hout = dr.tile([PB, E], BF16, tag="hout", name="hout")
    hgat = dr.tile([2 * PB, E], BF16, tag="hgat", name="hgat")
    nc.gpsimd.dma_start(hout[:], h[:])
    if stage == 72:      # timing experiment: no collective
        nc.gpsimd.dma_start(hgat[0:PB, :], hout[:])
    else:
        nc.gpsimd.collective_compute(
            "AllGather", OP.bypass,
            replica_groups=[[0, 4], [1, 5], [2, 6], [3, 7]],
            ins=[hout.opt()], outs=[hgat.opt()])
    hgatS = ap_.tile([2 * PB, E], BF16, tag="hgatS")
    nc.sync.dma_start(hgatS[:], hgat[:])

    pe_fill.flush()
    dve_fill.flush()

    if stage == 6:   # + GRU
        _dbg_out(h[0:1, 0:TB])
        return

    CT = ("mm", "ctxp")
    # ---------------- uncertainty features -> uf
    uf = fp.tile([128, NCH, 4 * K], BF16, tag="uf")  # feature row = k*4 + ci
    rp = fp.tile([128, NCH, 17], F32, tag="rootpack")  # 0:6 md, 6:11 cp, 11:17 sp
    rq = fp.tile([128, NCH, 17], F32, tag="rootout")

    # min_dist: d2(k,j) = n_k + n_j - 2 g_kj; min over partners; sqrt
    npair = fp.tile([128, NCH, NP], F32, tag="npair")
    for i, (k, j) in enumerate(PAIRS):
        nc.vector.tensor_tensor(npair[:, :, i], Nt[:, :, k], Nt[:, :, j], OP.add)
    d2 = fp.tile([128, NCH, NP], F32, tag="d2")
    nc.vector.scalar_tensor_tensor(out=d2[:], in0=Gt[:], scalar=-2.0,
                                   in1=npair[:], op0=OP.mult, op1=OP.add)
    pidx = {}
    for i, (k, j) in enumerate(PAIRS):
        pidx[(k, j)] = i
        pidx[(j, k)] = i
    md = rp[:, :, 0:K]
    for k in range(K):
        parts = [pidx[(k, j)] for j in range(K) if j != k]
        nc.vector.tensor_tensor(md[:, :, k], d2[:, :, parts[0]],
                                d2[:, :, parts[1]], OP.min)
        for i in parts[2:]:
            nc.vector.tensor_tensor(md[:, :, k], md[:, :, k], d2[:, :, i], OP.min)
    nc.vector.tensor_scalar_max(md, md, 0.0)

    # cos_dist (k>0) needs rsqrt(n_0 * n_k); sentence needs rsqrt(ns * n_k)
    cprod = rp[:, :, K:K + 5]
    for c in range(NCH):
        nc.vector.tensor_scalar(out=cprod[:, c, :], in0=Nt[:, c, 1:],
                                scalar1=Nt[:, c, 0:1], scalar2=None, op0=OP.mult)
    sprod = rp[:, :, K + 5:K + 11]
    nc.vector.tensor_scalar(out=sprod, in0=Nt[:], scalar1=Ns[:, 0:1],
                            scalar2=None, op0=OP.mult)
    nc.vector.reciprocal(rp[:, :, K:K + 11], rp[:, :, K:K + 11])
    nc.scalar.activation(rq[:], rp[:], AF.Sqrt)   # the only sqrt-set op

    nc.vector.tensor_copy(uf[:, :, 0::4], rq[:, :, 0:K])
    nc.vector.tensor_tensor(uf[:, :, 5:24:4], Gt[:, :, 0:K - 1],
                            rq[:, :, K:K + 5], OP.mult)
    nc.vector.memset(uf[:, :, 1], 0.0)
    nc.vector.tensor_tensor(uf[:, :, 2::4], St[:], rq[:, :, K + 5:K + 11], OP.mult)
    nc.vector.tensor_copy(uf[:, :, 3::4], probs[:])

    if stage == 3:   # + uncertainty features
        _dbg_out(uf[0:1, :, :].rearrange("p c k -> p (c k)"), n=NCH * 24)
        return

    # ---------------- conv windows via PE transpose, then the CNN
    win = []
    for l in range(4):
        wps = ps_.tile([12, TBP], BF16, tag="tp", name="win_ps")
        for c in range(NCH):
            nc.tensor.transpose(wps[0:12, c * 128:(c + 1) * 128],
                                uf[:, c, 4 * l:4 * l + 12], ident[:])
        t = fp.tile([12, TBP], BF16, tag=f"win{l}", name=f"win{l}")
        nc.vector.tensor_copy(t[:], wps[:])
        win.append(t)

    c1 = []
    for l in range(4):
        ps = ps_.tile([128, TBP], F32, tag=CT[l % 2], name="c1_ps")
        nc.tensor.matmul(ps[:], w1T[:], win[l][:], start=True, stop=True)
        c1.append(mm_epilogue(ps[:], b_c1, 0, f"c1_{l}", n=TBP))
    c2 = []
    for lp in range(2):
        for mc in range(2):
            ps = ps_.tile([128, TBP], F32, tag=CT[mc % 2], name="c2_ps")
            for dk in range(3):
                nc.tensor.matmul(ps[:], w2T[dk][:, mc * 128:(mc + 1) * 128],
                                 c1[lp + dk][:], start=(dk == 0), stop=(dk == 2))
            c2.append(mm_epilogue(ps[:], b_c2, mc, f"c2_{lp}{mc}", n=TBP))
    unc = []
    for mc in range(4):
        ps = ps_.tile([128, TBP], F32, tag=CT[mc % 2], name="c3_ps")
        for kk in range(4):
            nc.tensor.matmul(ps[:], w3T[kk][:, mc * 128:(mc + 1) * 128],
                             c2[kk][:], start=(kk == 0), stop=(kk == 3))
        unc.append(mm_epilogue(ps[:], b_c3, mc, f"unc{mc}", n=TBP))

    if stage == 4:   # + CNN
        _dbg_out(unc[0][0:1, 0:TB])
        return

    # held-back fillers: more PE work in the collective's shadow
    do_img(2)()
    do_img(3)()
    do_hid()

    # ---------------- own-rows h_f/h_b select + transpose for ctx
    psf = ps_.tile([128, 512], F32, tag="mm", name="hfin_ps")
    nc.tensor.matmul(psf[0:PB, :], hsel[:], hgatS[:], start=True, stop=True)
    hfin = ap_.tile([PB, E], BF16, tag="hfin")   # rows 0:16 h_f, 16:32 h_b
    nc.scalar.copy(hfin[:], psf[0:PB, :])
    hTfin = ap_.tile([128, 4, PB], BF16, tag="hTfin")  # cols 0:16 f, 16:32 b
    tpH = ps_.tile([128, 4, PB], BF16, tag="tp", name="tpH")
    for cc in range(4):
        nc.tensor.transpose(tpH[:, cc, :], hfin[0:PB, cc * 128:(cc + 1) * 128],
                            ident[0:PB, 0:PB])
    nc.vector.tensor_copy(hTfin[:], tpH[:])

    # ---------------- ctx = prelu(concat @ cW.T + cb), then logits
    # cW K-chunk order: cap_f(4) cap_b(4) pos(2) img(4) hid(2); accumulate
    # the noncap chunks (8..15) first so they run pre-exchange
    rhs_ctx = [hTfin[:, cc, 16 * di:16 * di + 16].unsqueeze(1)
               .broadcast_to([128, T, BPC]) for di in range(2) for cc in range(4)]
    rhs_ctx += [t[:] for t in posf + imgf + hidf]
    ORDER = list(range(8, 16)) + list(range(8))
    ctxa = [None] * 4
    for mca, mcb in ((0, 1), (2, 3)):
        pspair = {}
        for mc in (mca, mcb):
            cps = ps_.tile([128, TB], F32, tag=CT[mc % 2], name="ctx_ps")
            for i, kk in enumerate(ORDER[:8]):
                nc.tensor.matmul(cps[:], cWT[kk][:, mc * 128:(mc + 1) * 128],
                                 rhs_ctx[kk], start=(i == 0), stop=False)
            pspair[mc] = cps
        for mc in (mca, mcb):
            cps = pspair[mc]
            for i, kk in enumerate(ORDER[8:]):
                nc.tensor.matmul(cps[:], cWT[kk][:, mc * 128:(mc + 1) * 128],
                                 rhs_ctx[kk], start=False, stop=(i == 7))
            ctxa[mc] = mm_epilogue(cps[:], b_cb, mc, f"ctxa{mc}")

    psl = ps_.tile([1, TB], F32, tag="mm", name="lg_ps")
    rhs_o = [t[:, 0:TB] for t in unc] + [t[:] for t in ctxa]  # unc first
    for kk in range(8):
        nc.tensor.matmul(psl[:], oWT[kk][:], rhs_o[kk],
                         start=(kk == 0), stop=(kk == 7))
    lg = fp.tile([1, TB], F32, tag="lg")
    nc.scalar.copy(lg[:], psl[:])
    nc.sync.dma_start(io["out_logits"][:], lg[:])


# ---------------------------------------------------------------- build

@functools.lru_cache(maxsize=8)
def _build(stage=99):
    nc = bacc.Bacc("TRN2", target_bir_lowering=False, debug=False,
                   enable_asserts=False, num_devices=NCORE)
    io = {}

    def din(name, shape, dt):
        io[name] = nc.dram_tensor(name, list(shape), dt, kind="ExternalInput").ap()

    din("attT", [128, 16 * TB], BF16)
    din("hidT", [128, 4 * TB], BF16)
    din("posT", [51, TB], BF16)
    din("probsP", [TBP, K], F32)
    din("sentB", [128, E], BF16)
    din("ident", [128, 128], BF16)
    din("smallp", [1, G3 + 128], BF16)
    din("capeT", [128, 4 * TBP2], BF16)
    din("embP", [128, K * NCH * E], BF16)
    din("hselM", [2 * PB, PB], BF16)
    din("iWT", [128, 16 * 512], BF16)
    din("cWT", [128, 16 * 512], BF16)
    din("hWT", [128, 4 * 256], BF16)
    din("pWT", [128, 256], BF16)
    din("posW", [51, 128], BF16)
    din("w1T", [12, 128], BF16)
    din("w2T", [128, 3 * 256], BF16)
    din("w3T", [128, 4 * 512], BF16)
    din("oWT", [128, 8], BF16)
    din("wihT", [128, 4 * G3], BF16)
    din("whhT", [128, 4 * G3], BF16)
    din("biasP", [128, 19], F32)
    io["out_logits"] = nc.dram_tensor("out_logits", [1, TB], F32,
                                      kind="ExternalOutput").ap()

    with tile.TileContext(nc) as tc:
        _emit(nc, tc, io, stage)
    nc.compile()
    return nc


# ---------------------------------------------------------------- host side

def _bf(x):
    return np.ascontiguousarray(np.asarray(x, np.float32).astype(BF))


def _pmaj(a, k):
    """[k*128, n] -> [128, k*n] partition-major interleave (one-DMA layout)."""
    a = np.asarray(a)
    return np.ascontiguousarray(
        a.reshape(k, 128, -1).transpose(1, 0, 2).reshape(128, -1))


def _prep_core(ci, inp, shared):
    sl = slice(ci * BPC, (ci + 1) * BPC)
    attT = _pmaj(_bf(np.asarray(inp["attended_img"])[sl]
                     .transpose(2, 1, 0).reshape(IC, TB)), 16)
    hidT = _pmaj(_bf(np.asarray(inp["hidden"])[sl]
                     .transpose(2, 1, 0).reshape(512, TB)), 4)
    posT = _bf(np.asarray(inp["pos"])[sl].transpose(2, 1, 0).reshape(51, TB))
    probsP = np.zeros((TBP, K), np.float32)
    probsP[:TB] = np.asarray(inp["topk_probs"])[:, sl, :].reshape(TB, K)
    tw = np.asarray(inp["topk_words"])[:, sl, :].astype(np.int64)  # [21, 16, 6]

    # embP: topk-word embeddings, host-gathered. col j = k*NCH+c holds
    # emb[tw[tb]] for tb rows c*128..(c+1)*128 (tb = t*16+b, zero-padded)
    emb_bf = shared["weights"]["cap_emb_bf"]
    embP = np.zeros((128, K * NCH, E), BF)
    for k in range(K):
        col = np.zeros(TBP, np.int64)
        col[:TB] = tw[:, :, k].reshape(TB)
        for c in range(NCH):
            embP[:, k * NCH + c, :] = emb_bf[col[c * 128:(c + 1) * 128]]
    embP = embP.reshape(128, -1)

    # capeT: caption embeddings for this core's GRU direction over the 32
    # pair-batch rows, transposed to [128, 4, 768] (tb2 = t*32 + b2)
    lo = ci % 4
    caption = np.asarray(inp["caption"]).astype(np.int64)
    cap2 = np.concatenate([caption[lo * BPC:(lo + 1) * BPC],
                           caption[(lo + 4) * BPC:(lo + 5) * BPC]], axis=0)
    order = cap2.T.reshape(TB2) if ci < 4 else cap2[:, ::-1].T.reshape(TB2)
    ordp = np.zeros(TBP2, np.int64)
    ordp[:TB2] = order
    capW = shared["weights"]["cap_emb_w_bf"]
    capeT = _pmaj(np.ascontiguousarray(capW[ordp].T), 4)   # [128, 4*768]

    # h-exchange select matrix: hfin[p] = hgat[idx[p]]
    p = 0 if ci < 4 else 1
    idx = np.concatenate([np.arange(BPC) + BPC * p,
                          np.arange(BPC) + PB + BPC * p])
    hselM = np.zeros((2 * PB, PB), BF)
    hselM[idx, np.arange(PB)] = 1

    d = "f" if ci < 4 else "b"
    m = {
        "attT": attT, "hidT": hidT, "posT": posT, "probsP": probsP,
        "sentB": np.ascontiguousarray(np.tile(shared["sent"][sl], (8, 1))),
        "embP": embP, "capeT": capeT, "hselM": hselM,
        "wihT": shared["weights"][f"wihT_{d}"],
        "whhT": shared["weights"][f"whhT_{d}"],
        "smallp": shared["weights"][f"smallp_{d}"],
    }
    for k, v in shared["weights"].items():
        if not (k.startswith("wihT_") or k.startswith("whhT_")
                or k.startswith("smallp_") or k.startswith("cap_emb")):
            m[k] = v
    return m


def _prep_shared(inp):
    cap_emb = np.asarray(inp["cap_embedding"], np.float32)
    capt = np.asarray(inp["caption"]).astype(np.int64)
    cap_len = np.asarray(inp["cap_len"]).astype(np.int64)
    mask = (np.arange(T)[None, :] < cap_len[:, None]).astype(np.float32)
    sent = np.einsum("bte,bt->be", cap_emb[capt], mask)       # [B, E]

    w = {}
    w["cap_emb_bf"] = _bf(cap_emb)
    w["cap_emb_w_bf"] = _bf(inp["cap_emb_w"])
    w["iWT"] = _pmaj(_bf(np.asarray(inp["iW"], np.float32).T), 16)
    w["cWT"] = _pmaj(_bf(np.asarray(inp["cW"], np.float32).T), 16)
    w["hWT"] = _pmaj(_bf(np.asarray(inp["hW"], np.float32).T), 4)
    w["pWT"] = _bf(np.asarray(inp["pW"], np.float32).T)
    w["posW"] = _bf(inp["pos_emb_w"])
    w["w1T"] = _bf(np.asarray(inp["conv1_w"], np.float32).transpose(2, 1, 0).reshape(12, 128))
    w["w2T"] = _pmaj(_bf(np.asarray(inp["conv2_w"], np.float32).transpose(2, 1, 0).reshape(384, 256)), 3)
    w["w3T"] = _pmaj(_bf(np.asarray(inp["conv3_w"], np.float32).transpose(2, 1, 0).reshape(512, 512)), 4)
    # oWT packed in accumulation order [unc(4) | ctx(4)]
    oW = np.asarray(inp["oW"], np.float32).reshape(-1)
    oPack = np.concatenate([oW[512:1024], oW[0:512]])
    w["oWT"] = np.ascontiguousarray(_bf(oPack).reshape(8, 128).T)
    ones = np.ones((128,), np.float32)
    for d, sfx in (("f", "_f"), ("b", "_b")):
        w[f"wihT_{d}"] = _pmaj(_bf(np.asarray(inp["gru_w_ih" + sfx], np.float32).T), 4)
        w[f"whhT_{d}"] = _pmaj(_bf(np.asarray(inp["gru_w_hh" + sfx], np.float32).T), 4)
        bias = np.asarray(inp["gru_b_ih" + sfx], np.float32) \
            + np.asarray(inp["gru_b_hh" + sfx], np.float32)
        w[f"smallp_{d}"] = _bf(np.concatenate([ones, bias]).reshape(1, -1))
    bias_cols = []
    for key in ("ib", "pb", "hb", "cb", "conv1_b", "conv2_b", "conv3_b"):
        bias_cols.append(np.asarray(inp[key], np.float32).reshape(-1, 128).T)
    w["biasP"] = np.ascontiguousarray(np.concatenate(bias_cols, axis=1))
    w["ident"] = _bf(np.eye(128, dtype=np.float32))
    return {"weights": w, "sent": _bf(sent).astype(BF)}


_RUNNER = {}


def _get_runner():
    if "fn" not in _RUNNER:
        nc = _build()
        _RUNNER["nc"] = nc
        _RUNNER["fn"] = lambda in_maps: run_bass_kernel_spmd(
            nc, in_maps, core_ids=list(range(NCORE)))
    return _RUNNER["fn"]


def kernel(**inputs):
    fn = _get_runner()
    shared = _prep_shared(inputs)
    in_maps = [_prep_core(ci, inputs, shared) for ci in range(NCORE)]
    res = fn(in_maps)
    logits = np.zeros((B, T), np.float32)
    for ci in range(NCORE):
        lg = np.asarray(res.results[ci]["out_logits"], np.float32).reshape(TB)
        logits[ci * BPC:(ci + 1) * BPC] = lg.reshape(T, BPC).T
    logits += float(np.asarray(inputs["ob"]).reshape(-1)[0])
    pos = np.asarray(inputs["pos"])
    valid_pos = np.argmax(pos, axis=2) != (pos.shape[-1] - 2)
    return logits, valid_pos
